# revision 1
# baseline (speedup 1.0000x reference)
"""Trainium2 Bass kernel for Conv2D(sum of 20 1x1 convs) + QwenRMSNorm.

Math: y = einsum("bsi,loi->bso", x, conv_w) / L ; out = rmsnorm(y) * norm_w.
Since x does not depend on l, the 20-matrix contraction collapses to a single
matmul with W = sum_l conv_w[l] / L.  Host pre-sums/transposes/casts the weight
(one [H,H] matrix) and lays out x as token-sharded, hidden-major bf16 slabs;
the 8 NeuronCores each run matmul (bf16, fp32 accum) + RMSNorm on their 2048
tokens.  All device compute is token-local; no collectives.
"""

import numpy as np
import ml_dtypes
from contextlib import ExitStack

import concourse.bass as bass
import concourse.mybir as mybir
import concourse.tile as tile
from concourse.bass_utils import run_bass_kernel_spmd

N_CORES = 8
B, S, H, L = 4, 4096, 1024, 20
TOK = B * S               # 16384 tokens
TPC = TOK // N_CORES      # 2048 tokens per core
TB = TPC // 128           # 16 token-blocks of 128 per core
KB = H // 128             # 8 contraction blocks
NOH = H // 512            # 2 psum halves of the output row
EPS = 1e-6

BF16 = mybir.dt.bfloat16
F32 = mybir.dt.float32
AF = mybir.ActivationFunctionType
OP = mybir.AluOpType

_BUILT = None       # cached Bass program
LAST_RESULTS = None  # BassKernelResults of the most recent run (for test harness)


def _legalize_multiwait(nc):
    """The walrus build here encodes exactly one semaphore wait per 64B
    instruction (NEURON_ISA_TPB_EVENTS has a single wait slot) and errors on
    Tile's multi-wait instructions.  Split surplus waits into standalone
    EVENT_SEMAPHORE instructions on the same engine, placed directly before
    the original instruction (same sequencer stream -> same semantics)."""
    n_ev = 0
    for f in nc.m.functions:
        for blk in f.blocks:
            insts = blk.instructions
            out = []
            changed = False
            for inst in list(insts):
                si = getattr(inst, "sync_info", None)
                waits = list(si.on_wait) if si is not None else []
                if len(waits) > 1:
                    changed = True
                    updates = list(si.on_update)
                    for w in waits[:-1]:
                        ev = mybir.InstEventSemaphore(
                            name=f"{inst.name}-sw{n_ev}", ins=[], outs=[])
                        n_ev += 1
                        ev.engine = inst.engine
                        ev.sync_info = mybir.SyncInfo(on_wait=[w], on_update=[])
                        out.append(ev)
                    inst.sync_info = mybir.SyncInfo(
                        on_wait=[waits[-1]], on_update=updates)
                out.append(inst)
            if changed:
                insts.clear()
                insts.extend(out)


def _build(loop_k=1):
    nc = bass.Bass()
    # x^T slab layout per core: xt[tt, p, ib, t] = x[tt*128 + t, ib*128 + p], bf16
    xt_h = nc.dram_tensor("xt", [TB, 128, KB, 128], BF16, kind="ExternalInput")
    # weight layout: wt[p, ib, o] = W[o, ib*128 + p] with W = sum_l conv_w[l]/L, bf16
    wt_h = nc.dram_tensor("wt", [128, KB, H], BF16, kind="ExternalInput")
    nw_h = nc.dram_tensor("nw", [H], F32, kind="ExternalInput")
    out_h = nc.dram_tensor("out", [TPC, H], F32, kind="ExternalOutput")

    with tile.TileContext(nc) as tc, ExitStack() as ctx:
        xpool = ctx.enter_context(tc.tile_pool(name="x", bufs=TB))
        wpool = ctx.enter_context(tc.tile_pool(name="w", bufs=1))
        cpool = ctx.enter_context(tc.tile_pool(name="consts", bufs=1))
        opool = ctx.enter_context(tc.tile_pool(name="out", bufs=4))
        spool = ctx.enter_context(tc.tile_pool(name="scratch", bufs=2))
        stats = ctx.enter_context(tc.tile_pool(name="stats", bufs=8))
        psum = ctx.enter_context(tc.tile_pool(name="psum", bufs=4, space="PSUM"))

        # weight slabs: one DMA per ib on the scalar HWDGE ring so the first
        # matmul only gates on slab ib=0; x slabs stream on the sync ring.
        # Issue the first-needed slabs (w ib0, x slab0) before everything else.
        w_sb = wpool.tile([128, KB, H], BF16)
        x_sb = [xpool.tile([128, KB, 128], BF16, name=f"xs{tt}", tag="xsb")
                for tt in range(TB)]
        nc.scalar.dma_start(out=w_sb[:, 0:1, :], in_=wt_h[:, 0:1, :])
        nc.sync.dma_start(out=x_sb[0], in_=xt_h[0])
        for ib in range(1, KB):
            nc.scalar.dma_start(out=w_sb[:, ib:ib + 1, :],
                                in_=wt_h[:, ib:ib + 1, :])
        x_dmas = [nc.sync.dma_start(out=x_sb[tt], in_=xt_h[tt])
                  for tt in range(1, TB)]

        # norm_w broadcast to 128 partitions; ordering-only dep pushes this
        # 512KB transfer behind the first x slabs (its first consumer is the
        # t-block-0 scale at ~10us, so it is not urgent)
        nw_sb = cpool.tile([128, H], F32)
        nc.gpsimd.dma_start(
            out=nw_sb, in_=bass.AP(tensor=nw_h, offset=0, ap=[[0, 128], [1, H]]))
        zero_sb = cpool.tile([128, 1], F32)
        nc.vector.memset(zero_sb, 0.0)
        eps_sb = cpool.tile([128, 1], F32)
        nc.vector.memset(eps_sb, EPS)


        for tt in [t for _ in range(loop_k) for t in range(TB)]:
            yp = psum.tile([128, H], F32)
            for oh in range(NOH):
                for ib in range(KB):
                    nc.tensor.matmul(
                        yp[:, oh * 512:(oh + 1) * 512],
                        x_sb[tt][:, ib, :],
                        w_sb[:, ib, oh * 512:(oh + 1) * 512],
                        start=(ib == 0),
                        stop=(ib == KB - 1),
                    )
            # sum of squares over the hidden axis (free axis) on ACT,
            # one op per psum half so each waits on a single PE group
            sq = spool.tile([128, H], BF16)
            half_sums = stats.tile([128, 2], F32)
            for oh in range(NOH):
                sl = slice(oh * 512, (oh + 1) * 512)
                nc.scalar.activation(out=sq[:, sl], in_=yp[:, sl],
                                     func=AF.Square, bias=zero_sb,
                                     accum_out=half_sums[:, oh:oh + 1])
            ssum = stats.tile([128, 1], F32)
            nc.vector.tensor_add(out=ssum, in0=half_sums[:, 0:1],
                                 in1=half_sums[:, 1:2])
            # std = sqrt(mean + eps); rstd = 1/std
            std = stats.tile([128, 1], F32)
            nc.scalar.activation(out=std, in_=ssum, func=AF.Sqrt,
                                 bias=eps_sb, scale=1.0 / H)
            rstd = stats.tile([128, 1], F32)
            nc.vector.reciprocal(out=rstd, in_=std)
            # out = (y * rstd) * norm_w
            o_sb = opool.tile([128, H], F32)
            for oh in range(NOH):
                sl = slice(oh * 512, (oh + 1) * 512)
                nc.vector.scalar_tensor_tensor(
                    out=o_sb[:, sl], in0=yp[:, sl], scalar=rstd,
                    in1=nw_sb[:, sl], op0=OP.mult, op1=OP.mult,
                )
            nc.scalar.dma_start(out=out_h[tt * 128:(tt + 1) * 128, :], in_=o_sb)

    _legalize_multiwait(nc)
    return nc


def host_prep(x, conv_w, norm_w):
    """Shard + lay out the full inputs into per-core device input maps."""
    bf16 = ml_dtypes.bfloat16

    # Collapse the 20 1x1 convs: W[o,i] = sum_l conv_w[l,o,i] / L
    w = np.asarray(conv_w).sum(axis=0) * (1.0 / L)          # [H(o), H(i)] f32
    # wt[p, ib, o] = W[o, ib*128+p]
    wt = np.ascontiguousarray(
        w.reshape(H, KB, 128).transpose(2, 1, 0).astype(bf16))
    nw = np.ascontiguousarray(np.asarray(norm_w), dtype=np.float32)

    x2d = np.asarray(x).reshape(TOK, H)
    xbf = x2d.astype(bf16)

    in_maps = []
    for c in range(N_CORES):
        xc = xbf[c * TPC:(c + 1) * TPC]                      # [TPC, H]
        # xt[tt, p, ib, t] = xc[tt*128+t, ib*128+p]
        xtc = np.ascontiguousarray(
            xc.reshape(TB, 128, KB, 128).transpose(0, 3, 2, 1))
        in_maps.append({"xt": xtc, "wt": wt, "nw": nw})
    return in_maps


def kernel(x, conv_w, norm_w):
    global _BUILT, LAST_RESULTS
    if _BUILT is None:
        _BUILT = _build()
    nc = _BUILT

    x = np.asarray(x)
    out_dtype = x.dtype
    in_maps = host_prep(x, conv_w, norm_w)

    res = run_bass_kernel_spmd(nc, in_maps, core_ids=list(range(N_CORES)))
    LAST_RESULTS = res

    out = np.concatenate([r["out"] for r in res.results], axis=0)
    return out.reshape(B, S, H).astype(out_dtype, copy=False)



# revision 4
# speedup vs baseline: 1.0565x; 1.0565x over previous
"""Trainium2 Bass kernel for Conv2D(sum of 20 1x1 convs) + QwenRMSNorm.

Math: y = einsum("bsi,loi->bso", x, conv_w) / L ; out = rmsnorm(y) * norm_w.
Since x does not depend on l, the 20-matrix contraction collapses to a single
matmul with W = sum_l conv_w[l] / L.  Host pre-sums/transposes/casts the weight
(one [H,H] matrix) and lays out x as token-sharded, hidden-major bf16 slabs;
the 8 NeuronCores each run matmul (bf16, fp32 accum) + RMSNorm on their 2048
tokens.  All device compute is token-local; no collectives.

v2 layout/schedule notes:
 - DRAM layouts are partition-outermost so multi-tile DMA chunks are
   contiguous per partition (x in 4 chunks, w in 4 chunks split across both
   HWDGE rings so the critical w+x0 fill gets full HBM bandwidth).
 - ~10 warm-up matmuls on a memset tile run during the DMA fill so the PE
   HAM clock-gate is at 8/8 before the first real matmul.
 - A dummy activation preloads the ACT function table during the fill.
 - Output is written bf16 (host upcasts); rel-err cost ~1e-3.
 - norm_w == 1 (the spec's fill) skips the [128,H] norm_w broadcast and the
   per-tile tensor_tensor multiply; a general variant handles arbitrary
   norm_w.
"""

import numpy as np
import ml_dtypes
from contextlib import ExitStack

import concourse.bass as bass
import concourse.mybir as mybir
import concourse.tile as tile
from concourse.bass_utils import run_bass_kernel_spmd

N_CORES = 8
B, S, H, L = 4, 4096, 1024, 20
TOK = B * S               # 16384 tokens
TPC = TOK // N_CORES      # 2048 tokens per core
TB = TPC // 128           # 16 token-blocks of 128 per core
KB = H // 128             # 8 contraction blocks
NOH = H // 512            # 2 psum halves of the output row
EPS = 1e-6
N_WARM = 10               # HAM warm-up matmuls (N=512) during the DMA fill

BF16 = mybir.dt.bfloat16
F32 = mybir.dt.float32
AF = mybir.ActivationFunctionType
OP = mybir.AluOpType

_BUILT = {}          # variant -> cached Bass program
LAST_RESULTS = None  # BassKernelResults of the most recent run (for test harness)


def _legalize_multiwait(nc):
    """The walrus build here encodes exactly one semaphore wait per 64B
    instruction (NEURON_ISA_TPB_EVENTS has a single wait slot) and errors on
    Tile's multi-wait instructions.  Split surplus waits into standalone
    EVENT_SEMAPHORE instructions on the same engine, placed directly before
    the original instruction (same sequencer stream -> same semantics)."""
    n_ev = 0
    for f in nc.m.functions:
        for blk in f.blocks:
            insts = blk.instructions
            out = []
            changed = False
            for inst in list(insts):
                si = getattr(inst, "sync_info", None)
                waits = list(si.on_wait) if si is not None else []
                if len(waits) > 1:
                    changed = True
                    updates = list(si.on_update)
                    for w in waits[:-1]:
                        ev = mybir.InstEventSemaphore(
                            name=f"{inst.name}-sw{n_ev}", ins=[], outs=[])
                        n_ev += 1
                        ev.engine = inst.engine
                        ev.sync_info = mybir.SyncInfo(on_wait=[w], on_update=[])
                        out.append(ev)
                    inst.sync_info = mybir.SyncInfo(
                        on_wait=[waits[-1]], on_update=updates)
                out.append(inst)
            if changed:
                insts.clear()
                insts.extend(out)


def _build(with_nw):
    nc = bass.Bass()
    # x layout (partition-outermost): xt[p, tt, ib, t] = x[tt*128+t, ib*128+p]
    xt_h = nc.dram_tensor("xt", [128, TB, KB, 128], BF16, kind="ExternalInput")
    # w layout: wt[p, oh, ib, j] = W[oh*512+j, ib*128+p], W = sum_l conv_w[l]/L
    wt_h = nc.dram_tensor("wt", [128, NOH, KB, 512], BF16, kind="ExternalInput")
    if with_nw:
        nw_h = nc.dram_tensor("nw", [H], F32, kind="ExternalInput")
    out_h = nc.dram_tensor("out", [TPC, H], BF16, kind="ExternalOutput")

    with tile.TileContext(nc) as tc, ExitStack() as ctx:
        xpool = ctx.enter_context(tc.tile_pool(name="x", bufs=1))
        wpool = ctx.enter_context(tc.tile_pool(name="w", bufs=1))
        cpool = ctx.enter_context(tc.tile_pool(name="consts", bufs=1))
        opool = ctx.enter_context(tc.tile_pool(name="out", bufs=4))
        spool = ctx.enter_context(tc.tile_pool(name="scratch", bufs=2))
        stats = ctx.enter_context(tc.tile_pool(name="stats", bufs=8))
        psum = ctx.enter_context(tc.tile_pool(name="psum", bufs=4, space="PSUM"))

        # const tiles first so the warm-up matmuls' source is ready early
        wu_sb = cpool.tile([128, 512], BF16)
        nc.vector.memset(wu_sb, 0.0)
        zero_sb = cpool.tile([128, 1], F32)
        nc.vector.memset(zero_sb, 0.0)
        eps_sb = cpool.tile([128, 1], F32)
        nc.vector.memset(eps_sb, EPS)

        x_sb = xpool.tile([128, TB, KB, 128], BF16)
        w_sb = wpool.tile([128, NOH, KB, 512], BF16)

        # DMA schedule: the critical fill (x tile0 + all of w) is split
        # across both HWDGE rings so it runs at full HBM bandwidth; bulk x
        # follows on the sync ring ahead of its consumption.
        nc.sync.dma_start(out=x_sb[:, 0:1, 0:4, :], in_=xt_h[:, 0:1, 0:4, :])
        nc.sync.dma_start(out=x_sb[:, 0:1, 4:8, :], in_=xt_h[:, 0:1, 4:8, :])
        nc.scalar.dma_start(out=w_sb[:, 0:1, 0:4, :], in_=wt_h[:, 0:1, 0:4, :])
        nc.scalar.dma_start(out=w_sb[:, 0:1, 4:8, :], in_=wt_h[:, 0:1, 4:8, :])
        nc.sync.dma_start(out=w_sb[:, 1:2, 0:4, :], in_=wt_h[:, 1:2, 0:4, :])
        nc.sync.dma_start(out=w_sb[:, 1:2, 4:8, :], in_=wt_h[:, 1:2, 4:8, :])
        nc.sync.dma_start(out=x_sb[:, 1:2], in_=xt_h[:, 1:2])
        nc.sync.dma_start(out=x_sb[:, 2:4], in_=xt_h[:, 2:4])
        nc.sync.dma_start(out=x_sb[:, 4:8], in_=xt_h[:, 4:8])
        nc.sync.dma_start(out=x_sb[:, 8:16], in_=xt_h[:, 8:16])

        if with_nw:
            # norm_w broadcast to 128 partitions (general path only)
            nw_sb = cpool.tile([128, H], F32)
            nc.gpsimd.dma_start(
                out=nw_sb,
                in_=bass.AP(tensor=nw_h, offset=0, ap=[[0, 128], [1, H]]))

        # preload the ACT function table (Square/Sqrt) during the fill
        dummy = stats.tile([128, 1], F32)
        nc.scalar.activation(out=dummy, in_=zero_sb, func=AF.Square,
                             bias=zero_sb)

        # HAM warm-up: keep the PE busy from ~7us until the real matmuls
        # start so the clock gate opens to 8/8 before real work.
        wp = psum.tile([128, H], F32, name="wp", tag="yp")
        for _ in range(N_WARM):
            nc.tensor.matmul(wp[:, 0:512], wu_sb[:, 0:128], wu_sb,
                             start=True, stop=True)

        for tt in range(TB):
            yp = psum.tile([128, H], F32, tag="yp")
            for oh in range(NOH):
                for ib in range(KB):
                    nc.tensor.matmul(
                        yp[:, oh * 512:(oh + 1) * 512],
                        x_sb[:, tt, ib, :],
                        w_sb[:, oh, ib, :],
                        start=(ib == 0),
                        stop=(ib == KB - 1),
                    )
            # sum of squares over the hidden axis (free axis) on ACT,
            # one op per psum half so each waits on a single PE group
            sq = spool.tile([128, H], BF16)
            half_sums = stats.tile([128, 2], F32)
            for oh in range(NOH):
                sl = slice(oh * 512, (oh + 1) * 512)
                nc.scalar.activation(out=sq[:, sl], in_=yp[:, sl],
                                     func=AF.Square, bias=zero_sb,
                                     accum_out=half_sums[:, oh:oh + 1])
            ssum = stats.tile([128, 1], F32)
            nc.vector.tensor_add(out=ssum, in0=half_sums[:, 0:1],
                                 in1=half_sums[:, 1:2])
            # std = sqrt(mean + eps); rstd = 1/std
            std = stats.tile([128, 1], F32)
            nc.scalar.activation(out=std, in_=ssum, func=AF.Sqrt,
                                 bias=eps_sb, scale=1.0 / H)
            rstd = stats.tile([128, 1], F32)
            nc.vector.reciprocal(out=rstd, in_=std)
            # out = (y * rstd) [* norm_w], written bf16
            o_sb = opool.tile([128, H], BF16)
            if with_nw:
                for oh in range(NOH):
                    sl = slice(oh * 512, (oh + 1) * 512)
                    nc.vector.scalar_tensor_tensor(
                        out=o_sb[:, sl], in0=yp[:, sl], scalar=rstd,
                        in1=nw_sb[:, sl], op0=OP.mult, op1=OP.mult,
                    )
            else:
                # split the two halves across ACT and DVE so they overlap
                nc.scalar.activation(out=o_sb[:, 0:512], in_=yp[:, 0:512],
                                     func=AF.Copy, scale=rstd)
                nc.vector.tensor_scalar_mul(out=o_sb[:, 512:1024],
                                            in0=yp[:, 512:1024],
                                            scalar1=rstd)
            nc.sync.dma_start(out=out_h[tt * 128:(tt + 1) * 128, :], in_=o_sb)

    _legalize_multiwait(nc)
    return nc


def host_prep(x, conv_w, norm_w, with_nw):
    """Shard + lay out the full inputs into per-core device input maps."""
    bf16 = ml_dtypes.bfloat16

    # Collapse the 20 1x1 convs: W[o,i] = sum_l conv_w[l,o,i] / L
    w = np.asarray(conv_w).sum(axis=0) * (1.0 / L)          # [H(o), H(i)] f32
    # wt[p, oh, ib, j] = W[oh*512+j, ib*128+p]
    wt = np.ascontiguousarray(
        w.reshape(NOH, 512, KB, 128).transpose(3, 0, 2, 1).astype(bf16))

    x2d = np.asarray(x).reshape(TOK, H)
    xbf = x2d.astype(bf16)

    in_maps = []
    for c in range(N_CORES):
        xc = xbf[c * TPC:(c + 1) * TPC]                      # [TPC, H]
        # xt[p, tt, ib, t] = xc[tt*128+t, ib*128+p]
        xtc = np.ascontiguousarray(
            xc.reshape(TB, 128, KB, 128).transpose(3, 0, 2, 1))
        m = {"xt": xtc, "wt": wt}
        if with_nw:
            m["nw"] = np.ascontiguousarray(np.asarray(norm_w),
                                           dtype=np.float32)
        in_maps.append(m)
    return in_maps


def kernel(x, conv_w, norm_w):
    global LAST_RESULTS

    x = np.asarray(x)
    out_dtype = x.dtype
    nw = np.asarray(norm_w)
    with_nw = not bool(np.all(nw == 1.0))

    if with_nw not in _BUILT:
        _BUILT[with_nw] = _build(with_nw)
    nc = _BUILT[with_nw]

    in_maps = host_prep(x, conv_w, norm_w, with_nw)

    res = run_bass_kernel_spmd(nc, in_maps, core_ids=list(range(N_CORES)))
    LAST_RESULTS = res

    out = np.concatenate([r["out"] for r in res.results], axis=0)
    return out.reshape(B, S, H).astype(out_dtype, copy=False)
